# revision 1
# baseline (speedup 1.0000x reference)
"""ChildSum TreeLSTM on a complete binary tree (131071 nodes, depth 17),
distributed over 8 trn2 NeuronCores.

Sharding: core k owns the subtree rooted at level-3 node (7+k): levels
16..CUT split contiguously 8 ways -> zero cross-core traffic. Host
computes levels CUT-1..0 in numpy (tiny, latency-bound on device).

Device layout: everything feature-major [feat(part), node(free)].  Each
level's nodes are stored in "children-split" order (s_3=[0];
s_{l+1}=[2i for i in s_l]+[2i+1 ...]) so a parent at stored pos j has its
left child at child-stored pos j and right child at pos n_parent+j ->
all child access is two contiguous slices.
"""
import sys
import numpy as np

for _p in ('/opt/trn_rl_repo',):
    if _p not in sys.path:
        sys.path.insert(0, _p)

N_NODES, D, P = 131071, 256, 128
NCORES = 8
CUT = int(__import__('os').environ.get('KERNEL_CUT', '14'))  # device: levels 16..CUT
LVLS = list(range(16, CUT - 1, -1))
CNT = {l: (2 ** l) // NCORES for l in LVLS}
SEG = {}
_off = 0
for _l in LVLS:
    SEG[_l] = _off
    _off += CNT[_l]
NLOC = _off            # 16256
NLOC_PAD = 16384
NT = 512               # node tile width


def stored_orders():
    s = {3: np.array([0], dtype=np.int64)}
    for l in range(3, 17):
        s[l + 1] = np.concatenate([2 * s[l], 2 * s[l] + 1])
    return s


_PROGRAM_CACHE = {}


def build_program(repeat=1, pool_off=False, fg_dedup=False, diag_act=0,
                  diag_dve=0, pool_split=0):
    key = ('nc', repeat, pool_off, fg_dedup, diag_act, diag_dve, pool_split)
    pool_off = pool_off or pool_split >= 1
    if key in _PROGRAM_CACHE:
        return _PROGRAM_CACHE[key]
    import concourse.bacc as bacc
    import concourse.mybir as mybir
    import concourse.tile as tile
    from contextlib import ExitStack, nullcontext

    f32 = mybir.dt.float32
    bf16 = mybir.dt.bfloat16
    AF = mybir.ActivationFunctionType

    nc = bacc.Bacc("TRN2", target_bir_lowering=False, debug=False,
                   num_devices=NCORES)

    x_d = nc.dram_tensor("x", [2, P, NLOC_PAD], bf16, kind="ExternalInput").ap()
    wx_d = nc.dram_tensor("wioux", [2, P, 768], bf16, kind="ExternalInput").ap()
    wh_d = nc.dram_tensor("wiouh", [2, P, 768], bf16, kind="ExternalInput").ap()
    wfx_d = nc.dram_tensor("wfx", [2, P, 256], bf16, kind="ExternalInput").ap()
    wfh_d = nc.dram_tensor("wfh", [2, P, 256], bf16, kind="ExternalInput").ap()
    bio_d = nc.dram_tensor("bio", [P, 6], f32, kind="ExternalInput").ap()
    bf_d = nc.dram_tensor("bf", [P, 2], f32, kind="ExternalInput").ap()
    outh_d = nc.dram_tensor("outh", [2, P, CNT[CUT]], bf16,
                            kind="ExternalOutput").ap()
    outc_d = nc.dram_tensor("outc", [2, P, CNT[CUT]], f32,
                            kind="ExternalOutput").ap()

    with tile.TileContext(nc) as tc, ExitStack() as ctx:
        wpool = ctx.enter_context(tc.tile_pool(name="w", bufs=1))
        hcpool = ctx.enter_context(tc.tile_pool(name="hc", bufs=1))
        xpool = ctx.enter_context(tc.tile_pool(name="xp", bufs=12))
        gpool = ctx.enter_context(tc.tile_pool(name="gp", bufs=1))
        ppool = ctx.enter_context(tc.tile_pool(name="pp", bufs=6, space="PSUM"))
        fpool = ctx.enter_context(tc.tile_pool(name="pf", bufs=1, space="PSUM"))

        # ---- weights / biases in SBUF (persistent) ----
        WX, WH, WFX, WFH = [], [], [], []
        for c in range(2):
            t = wpool.tile([P, 768], bf16, name=f"wxs{c}")
            nc.sync.dma_start(t[:], wx_d[c])
            WX.append(t)
            t = wpool.tile([P, 768], bf16, name=f"whs{c}")
            nc.sync.dma_start(t[:], wh_d[c])
            WH.append(t)
            t = wpool.tile([P, 256], bf16, name=f"wfxs{c}")
            nc.sync.dma_start(t[:], wfx_d[c])
            WFX.append(t)
            t = wpool.tile([P, 256], bf16, name=f"wfhs{c}")
            nc.sync.dma_start(t[:], wfh_d[c])
            WFH.append(t)
        BIO = wpool.tile([P, 6], f32, name="bios")
        nc.sync.dma_start(BIO[:], bio_d[:])
        BF = wpool.tile([P, 2], f32, name="bfs")
        nc.sync.dma_start(BF[:], bf_d[:])

        # ---- persistent H/C buffers: lvl14 + ping-pong for 13..3 ----
        H14 = [hcpool.tile([P, 2048], bf16, name=f"H14_{c}") for c in range(2)]
        C14 = [hcpool.tile([P, 2048], f32, name=f"C14_{c}") for c in range(2)]
        HA = [hcpool.tile([P, 1024], bf16, name=f"HA{c}") for c in range(2)]
        CA = [hcpool.tile([P, 1024], f32, name=f"CA{c}") for c in range(2)]
        HB = [hcpool.tile([P, 512], bf16, name=f"HB{c}") for c in range(2)]
        CB = [hcpool.tile([P, 512], f32, name=f"CB{c}") for c in range(2)]

        def bufs_for(lvl):
            if lvl == 14:
                return (H14, C14)
            return (HA, CA) if lvl % 2 == 1 else (HB, CB)

        loop_cm = tc.For_i(0, repeat, 1) if repeat > 1 else nullcontext()

        def load_x(lvl, j, m):
            xs = []
            for c in range(2):
                xt = xpool.tile([P, NT], bf16, name="xt", tag="xt")
                nc.sync.dma_start(xt[:, :m],
                                  x_d[c, :, SEG[lvl] + j: SEG[lvl] + j + m])
                xs.append(xt)
            return xs

        def diag_load(m):
            for _ in range(diag_act):
                d = gpool.tile([P, NT], f32, name="dmp", tag="dmp", bufs=2)
                nc.scalar.activation(d[:, :m], WX[0][:, 0:m], AF.Sigmoid,
                                     bias=BIO[:, 0:1])
            for _ in range(diag_dve):
                d = gpool.tile([P, NT], f32, name="dmv", tag="dmv", bufs=2)
                nc.vector.tensor_add(d[:, :m], WFX[0][:, 0:m],
                                     WFX[1][:, 0:m])

        def unit_leaf(xs, m, houts, couts):
            """leaf recurrence for m nodes; writes into houts/couts AP slices"""
            diag_load(m)
            gates = []
            for fo in range(6):
                pt = ppool.tile([P, NT], f32, name="pt", tag="pt")
                nc.tensor.matmul(pt[:, :m], WX[0][:, fo * P:(fo + 1) * P],
                                 xs[0][:, :m], start=True, stop=False)
                nc.tensor.matmul(pt[:, :m], WX[1][:, fo * P:(fo + 1) * P],
                                 xs[1][:, :m], start=False, stop=True)
                g = gpool.tile([P, NT], f32, name="lg", tag=f"lg{fo}", bufs=2)
                func = AF.Tanh if fo >= 4 else AF.Sigmoid
                nc.scalar.activation(g[:, :m], pt[:, :m], func,
                                     bias=BIO[:, fo:fo + 1])
                gates.append(g)
            for c in range(2):
                ct = couts(c)
                engc = nc.gpsimd if pool_split >= 2 else nc.vector
                engc.tensor_mul(ct, gates[c][:, :m], gates[4 + c][:, :m])
                tt = gpool.tile([P, NT], f32, name="lt", tag=f"tmp{c}")
                nc.scalar.activation(tt[:, :m], ct, AF.Tanh)
                nc.vector.tensor_mul(houts(c), gates[2 + c][:, :m], tt[:, :m])

        def unit_internal(xs, hL, cL, hR, cR, houts, couts, m):
            """internal recurrence for m nodes; children/outputs via callables"""
            diag_load(m)
            hsum = []
            for c in range(2):
                hs = gpool.tile([P, NT], bf16, name="hs", tag=f"hs{c}")
                if pool_off:
                    nc.gpsimd.tensor_add(hs[:, :m], hL(c), hR(c))
                else:
                    nc.vector.tensor_add(hs[:, :m], hL(c), hR(c))
                hsum.append(hs)
            gates = []
            for fo in range(6):
                pt = ppool.tile([P, NT], f32, name="pt", tag="pt")
                nc.tensor.matmul(pt[:, :m], WX[0][:, fo * P:(fo + 1) * P],
                                 xs[0][:, :m], start=True, stop=False)
                nc.tensor.matmul(pt[:, :m], WX[1][:, fo * P:(fo + 1) * P],
                                 xs[1][:, :m], start=False, stop=False)
                nc.tensor.matmul(pt[:, :m], WH[0][:, fo * P:(fo + 1) * P],
                                 hsum[0][:, :m], start=False, stop=False)
                nc.tensor.matmul(pt[:, :m], WH[1][:, fo * P:(fo + 1) * P],
                                 hsum[1][:, :m], start=False, stop=True)
                g = gpool.tile([P, NT], f32, name="ig", tag=f"ig{fo}", bufs=2)
                func = AF.Tanh if fo >= 4 else AF.Sigmoid
                nc.scalar.activation(g[:, :m], pt[:, :m], func,
                                     bias=BIO[:, fo:fo + 1])
                gates.append(g)
            # forget gates: [fL | fR] share one 2-bank psum tile per fo chunk
            # (same per-partition bias and same sigmoid for both halves).
            # The shared x-projection W_fx@x is computed once per fo chunk
            # and DVE-added into both halves (saves 4 matmuls per unit).
            fg = []
            for fo in range(2):
                pf = fpool.tile([P, 2 * NT], f32, name="pf", tag="pf")
                if fg_dedup:
                    pfx = ppool.tile([P, NT], f32, name="pt", tag="pt")
                    nc.tensor.matmul(pfx[:, :m],
                                     WFX[0][:, fo * P:(fo + 1) * P],
                                     xs[0][:, :m], start=True, stop=False)
                    nc.tensor.matmul(pfx[:, :m],
                                     WFX[1][:, fo * P:(fo + 1) * P],
                                     xs[1][:, :m], start=False, stop=True)
                    # DVE has a single PSUM read port: stage fx in SBUF
                    # before adding it into the fg psum halves
                    fxs = gpool.tile([P, NT], f32, name="fxs", tag=f"fxs{fo}")
                    nc.vector.tensor_copy(fxs[:, :m], pfx[:, :m])
                for half, hh in ((0, hL), (1, hR)):
                    sl = slice(half * m, half * m + m)
                    nc.tensor.matmul(pf[:, sl],
                                     WFH[0][:, fo * P:(fo + 1) * P],
                                     hh(0), start=True, stop=False)
                    if fg_dedup:
                        nc.tensor.matmul(pf[:, sl],
                                         WFH[1][:, fo * P:(fo + 1) * P],
                                         hh(1), start=False, stop=True)
                        nc.vector.tensor_add(pf[:, sl], pf[:, sl],
                                             fxs[:, :m])
                    else:
                        nc.tensor.matmul(pf[:, sl],
                                         WFH[1][:, fo * P:(fo + 1) * P],
                                         hh(1), start=False, stop=False)
                        nc.tensor.matmul(pf[:, sl],
                                         WFX[0][:, fo * P:(fo + 1) * P],
                                         xs[0][:, :m], start=False, stop=False)
                        nc.tensor.matmul(pf[:, sl],
                                         WFX[1][:, fo * P:(fo + 1) * P],
                                         xs[1][:, :m], start=False, stop=True)
                g = gpool.tile([P, 2 * NT], f32, name="fgt", tag=f"fgt{fo}")
                nc.scalar.activation(g[:, :2 * m], pf[:, :2 * m], AF.Sigmoid,
                                     bias=BF[:, fo:fo + 1])
                fg.append(g)
            for c in range(2):
                t1 = gpool.tile([P, NT], f32, name="t1", tag=f"t1{c}")
                eng1 = nc.gpsimd if pool_split >= 3 else nc.vector
                eng1.tensor_mul(t1[:, :m], fg[c][:, :m], cL(c))
                t2 = gpool.tile([P, NT], f32, name="t2", tag=f"t2{c}")
                if pool_off:
                    nc.gpsimd.tensor_mul(t2[:, :m], fg[c][:, m:2 * m], cR(c))
                else:
                    nc.vector.tensor_mul(t2[:, :m], fg[c][:, m:2 * m], cR(c))
                nc.vector.tensor_add(t1[:, :m], t1[:, :m], t2[:, :m])
                t3 = gpool.tile([P, NT], f32, name="t3", tag=f"t3{c}")
                eng3 = nc.gpsimd if pool_split >= 2 else nc.vector
                eng3.tensor_mul(t3[:, :m], gates[c][:, :m],
                                gates[4 + c][:, :m])
                cn = couts(c)
                enga = nc.gpsimd if pool_split >= 3 else nc.vector
                enga.tensor_add(cn, t3[:, :m], t1[:, :m])
                tt = gpool.tile([P, NT], f32, name="tt", tag=f"tt{c}")
                nc.scalar.activation(tt[:, :m], cn, AF.Tanh)
                engh = nc.gpsimd if pool_split >= 2 else nc.vector
                engh.tensor_mul(houts(c), gates[2 + c][:, :m], tt[:, :m])

        def sl_w(bufs, c, j, m):
            return bufs[c][:, j:j + m]

        with loop_cm:
            # ---- phase 1: leaves + lvl15 fused into lvl14 tiles ----
            for t in range(CNT[14] // NT):
                p15 = {}
                for u, s in (("a", t * NT), ("b", CNT[15] // 2 + t * NT)):
                    lhv, lcv = {}, {}
                    for S, js in (("L", s), ("R", CNT[16] // 2 + s)):
                        xsL = load_x(16, js, NT)
                        hts = [gpool.tile([P, NT], bf16, name="e16h", bufs=2,
                                          tag=f"e16h{S}{c}") for c in range(2)]
                        cts = [gpool.tile([P, NT], f32, name="e16c", bufs=2,
                                          tag=f"e16c{S}{c}") for c in range(2)]
                        unit_leaf(xsL, NT, lambda c: hts[c][:, :NT],
                                  lambda c: cts[c][:, :NT])
                        lhv[S], lcv[S] = hts, cts
                    xs15 = load_x(15, s, NT)
                    h15 = [gpool.tile([P, NT], bf16, name="p15h", bufs=2,
                                      tag=f"p15h{u}{c}") for c in range(2)]
                    c15 = [gpool.tile([P, NT], f32, name="p15c", bufs=2,
                                      tag=f"p15c{u}{c}") for c in range(2)]
                    unit_internal(xs15,
                                  lambda c: lhv["L"][c][:, :NT],
                                  lambda c: lcv["L"][c][:, :NT],
                                  lambda c: lhv["R"][c][:, :NT],
                                  lambda c: lcv["R"][c][:, :NT],
                                  lambda c: h15[c][:, :NT],
                                  lambda c: c15[c][:, :NT], NT)
                    p15[u] = (h15, c15)
                xs14 = load_x(14, t * NT, NT)
                j = t * NT
                unit_internal(xs14,
                              lambda c: p15["a"][0][c][:, :NT],
                              lambda c: p15["a"][1][c][:, :NT],
                              lambda c: p15["b"][0][c][:, :NT],
                              lambda c: p15["b"][1][c][:, :NT],
                              lambda c: H14[c][:, j:j + NT],
                              lambda c: C14[c][:, j:j + NT], NT)
                if CUT == 14:
                    for c in range(2):
                        nc.sync.dma_start(outh_d[c, :, j:j + NT],
                                          H14[c][:, j:j + NT])
                        nc.sync.dma_start(outc_d[c, :, j:j + NT],
                                          C14[c][:, j:j + NT])

            # ---- phase 2: levels 13..CUT ----
            for lvl in range(13, CUT - 1, -1):
                n = CNT[lvl]
                HC, CC = bufs_for(lvl + 1)
                HO, CO = bufs_for(lvl)
                for j in range(0, n, NT):
                    m = min(NT, n - j)
                    xsP = load_x(lvl, j, m)
                    unit_internal(
                        xsP,
                        lambda c: HC[c][:, j:j + m],
                        lambda c: CC[c][:, j:j + m],
                        lambda c: HC[c][:, n + j:n + j + m],
                        lambda c: CC[c][:, n + j:n + j + m],
                        lambda c: HO[c][:, j:j + m],
                        lambda c: CO[c][:, j:j + m], m)

            # ---- output: this core's level-CUT states ----
            if CUT < 14:
                HT, CT = bufs_for(CUT)
                nt = CNT[CUT]
                for c in range(2):
                    nc.sync.dma_start(outh_d[c], HT[c][:, 0:nt])
                    nc.sync.dma_start(outc_d[c], CT[c][:, 0:nt])

    nc.compile()
    _PROGRAM_CACHE[key] = nc
    return nc


def shard_inputs(inputs, W_ioux, b_ioux, W_iouh, b_iouh, W_fx, b_fx, W_fh, b_fh):
    """Build per-core input maps."""
    from ml_dtypes import bfloat16
    so = stored_orders()
    f32 = np.float32
    wioux = np.ascontiguousarray(
        np.asarray(W_ioux, f32).T.reshape(2, P, 768)).astype(bfloat16)
    wiouh = np.ascontiguousarray(
        np.asarray(W_iouh, f32).T.reshape(2, P, 768)).astype(bfloat16)
    wfx = np.ascontiguousarray(
        np.asarray(W_fx, f32).T.reshape(2, P, 256)).astype(bfloat16)
    wfh = np.ascontiguousarray(
        np.asarray(W_fh, f32).T.reshape(2, P, 256)).astype(bfloat16)
    bio = np.ascontiguousarray((np.asarray(b_ioux, f32)
                                + np.asarray(b_iouh, f32)).reshape(6, P).T)
    bf = np.ascontiguousarray((np.asarray(b_fx, f32)
                               + np.asarray(b_fh, f32)).reshape(2, P).T)
    inputs = np.asarray(inputs, f32)

    in_maps = []
    for k in range(NCORES):
        xk = np.zeros((NLOC_PAD, D), dtype=f32)
        for l in LVLS:
            n = CNT[l]
            gs = 2 ** l - 1 + k * n
            xk[SEG[l]:SEG[l] + n] = inputs[gs:gs + n][so[l]]
        xk = np.ascontiguousarray(xk.T).reshape(2, P, NLOC_PAD).astype(bfloat16)
        in_maps.append({
            "x": xk, "wioux": wioux, "wiouh": wiouh, "wfx": wfx, "wfh": wfh,
            "bio": bio, "bf": bf,
        })
    return in_maps


def _sig(v):
    return 1.0 / (1.0 + np.exp(-v))


def top_of_tree(h_cut, c_cut, inputs, W_ioux, b_ioux, W_iouh, b_iouh,
                W_fx, b_fx, W_fh, b_fh):
    """numpy levels CUT-1..0. h_cut/c_cut: [2^CUT, 256] level-CUT states."""
    f32 = np.float32
    n_top = 2 ** (CUT + 1) - 1
    ncut = 2 ** CUT
    h = np.zeros((n_top, D), dtype=f32)
    c = np.zeros((n_top, D), dtype=f32)
    h[ncut - 1:] = h_cut
    c[ncut - 1:] = c_cut
    x = np.asarray(inputs[:ncut - 1], f32)
    iou_x = x @ np.asarray(W_ioux, f32).T + b_ioux
    fx = x @ np.asarray(W_fx, f32).T + b_fx
    W_iouh = np.asarray(W_iouh, f32)
    W_fh = np.asarray(W_fh, f32)

    for lvl in range(CUT - 1, -1, -1):
        start, count = 2 ** lvl - 1, 2 ** lvl
        cs = 2 * start + 1
        ch = h[cs:cs + 2 * count].reshape(count, 2, D)
        cc = c[cs:cs + 2 * count].reshape(count, 2, D)
        iou = iou_x[start:start + count] + ch.sum(axis=1) @ W_iouh.T + b_iouh
        f = _sig(np.einsum("nkm,pm->nkp", ch, W_fh, optimize=True) + b_fh
                 + fx[start:start + count][:, None, :])
        fc_sum = (f * cc).sum(axis=1)
        i, o, u = np.split(iou, 3, axis=1)
        c_new = _sig(i) * np.tanh(u) + fc_sum
        h_new = _sig(o) * np.tanh(c_new)
        c[start:start + count] = c_new
        h[start:start + count] = h_new
    return c[0:1].astype(f32), h[0:1].astype(f32)


def run_device(in_maps, trace=False, repeat=1, pool_off=False):
    from concourse.bass_utils import run_bass_kernel_spmd
    nc = build_program(repeat, pool_off)
    return run_bass_kernel_spmd(nc, in_maps, core_ids=list(range(NCORES)),
                                trace=trace)


def kernel(inputs, W_ioux, b_ioux, W_iouh, b_iouh, W_fx, b_fx, W_fh, b_fh):
    args = (inputs, W_ioux, b_ioux, W_iouh, b_iouh, W_fx, b_fx, W_fh, b_fh)
    in_maps = shard_inputs(*args)
    res = run_device(in_maps)
    f32 = np.float32
    so = stored_orders()[CUT]
    nt = CNT[CUT]
    ncut = 2 ** CUT
    h_cut = np.zeros((ncut, D), dtype=f32)
    c_cut = np.zeros((ncut, D), dtype=f32)
    for k in range(NCORES):
        oh = res.results[k]["outh"]          # [2, P, nt] bf16
        oc = res.results[k]["outc"]          # [2, P, nt] f32
        idx = k * nt + so
        for c in range(2):
            h_cut[idx, c * P:(c + 1) * P] = np.asarray(oh[c], f32).T
            c_cut[idx, c * P:(c + 1) * P] = np.asarray(oc[c], f32).T
    return top_of_tree(h_cut, c_cut, *args)



# revision 2
# speedup vs baseline: 3.0973x; 3.0973x over previous
"""ChildSum TreeLSTM on a complete binary tree (131071 nodes, depth 17),
distributed over 8 trn2 NeuronCores.

Sharding: core k owns the subtree rooted at level-3 node (7+k): levels
16..CUT split contiguously 8 ways -> zero cross-core traffic. Host
computes levels CUT-1..0 in numpy (tiny, latency-bound on device).

Device layout: feature-major [feat(part), node(free)], with BOTH feature
halves folded into each tile: state tiles are [P, 2, n] where dim1 is the
feature half (feat = half*128 + partition).  Nodes within a level are in
"children-split" order (parent stored pos j has left child at child pos j,
right child at pos n_parent + j -> child access is contiguous slices).

Per-macro-tile (1024 nodes = 2 psum subtiles of 512):
  - x-side matmuls run in fp8e4m3 with DoubleRow perf mode (K=256 in one
    MM); h-side matmuls stay bf16 (K=128 x2).
  - psum gate tiles are [P, 2(sub), 512] so one ACT instruction covers
    1024 elems with a single per-partition bias (same fo chunk for both
    subtiles); forget gates use one [P, 2(sub), 2(LR), 512] psum tile per
    fo -> one 2048-wide sigmoid per fo.
  - the elementwise chain runs on DVE in bf16 (2x mode), both feature
    halves per instruction.
"""
import os
import sys
import numpy as np

for _p in ('/opt/trn_rl_repo',):
    if _p not in sys.path:
        sys.path.insert(0, _p)

N_NODES, D, P = 131071, 256, 128
NCORES = 8
CUT = int(os.environ.get('KERNEL_CUT', '14'))   # device computes levels 16..CUT
MACRO = 1024                                    # nodes per macro tile
SUB = 512                                       # psum subtile width
LVLS = list(range(16, CUT - 1, -1))
CNT = {l: (2 ** l) // NCORES for l in LVLS}
SEG = {}
_off = 0
for _l in LVLS:
    SEG[_l] = _off
    _off += CNT[_l]
NLOC = _off
NBLK = NLOC // MACRO


def stored_orders():
    s = {3: np.array([0], dtype=np.int64)}
    for l in range(3, 17):
        s[l + 1] = np.concatenate([2 * s[l], 2 * s[l] + 1])
    return s


_PROGRAM_CACHE = {}


def build_program(repeat=1):
    key = ('nc', repeat, CUT)
    if key in _PROGRAM_CACHE:
        return _PROGRAM_CACHE[key]
    import concourse.bacc as bacc
    import concourse.mybir as mybir
    import concourse.tile as tile
    from contextlib import ExitStack, nullcontext

    f32 = mybir.dt.float32
    bf16 = mybir.dt.bfloat16
    fp8 = mybir.dt.float8e4
    AF = mybir.ActivationFunctionType
    DR = mybir.MatmulPerfMode.DoubleRow

    nc = bacc.Bacc("TRN2", target_bir_lowering=False, debug=False,
                   num_devices=NCORES)

    x_d = nc.dram_tensor("x", [P, NBLK, 2, MACRO], fp8,
                         kind="ExternalInput").ap()
    wx_d = nc.dram_tensor("wioux", [P, 2, 768], fp8, kind="ExternalInput").ap()
    wfx_d = nc.dram_tensor("wfx", [P, 2, 256], fp8, kind="ExternalInput").ap()
    wh_d = nc.dram_tensor("wiouh", [2, P, 768], bf16,
                          kind="ExternalInput").ap()
    wfh_d = nc.dram_tensor("wfh", [2, P, 256], bf16, kind="ExternalInput").ap()
    bio_d = nc.dram_tensor("bio", [P, 6], f32, kind="ExternalInput").ap()
    bf_d = nc.dram_tensor("bf", [P, 2], f32, kind="ExternalInput").ap()
    outh_d = nc.dram_tensor("outh", [P, 2, CNT[CUT]], bf16,
                            kind="ExternalOutput").ap()
    outc_d = nc.dram_tensor("outc", [P, 2, CNT[CUT]], bf16,
                            kind="ExternalOutput").ap()

    with tile.TileContext(nc) as tc, ExitStack() as ctx:
        wpool = ctx.enter_context(tc.tile_pool(name="w", bufs=1))
        hcpool = ctx.enter_context(tc.tile_pool(name="hc", bufs=1))
        xpool = ctx.enter_context(tc.tile_pool(name="xp", bufs=3))
        gpool = ctx.enter_context(tc.tile_pool(name="gp", bufs=1))
        ppool = ctx.enter_context(tc.tile_pool(name="pp", bufs=2,
                                               space="PSUM"))
        fpool = ctx.enter_context(tc.tile_pool(name="pf", bufs=1,
                                               space="PSUM"))

        # ---- weights / biases in SBUF (persistent) ----
        WX = wpool.tile([P, 2, 768], fp8, name="wxs")
        nc.sync.dma_start(WX[:], wx_d[:])
        WFX = wpool.tile([P, 2, 256], fp8, name="wfxs")
        nc.sync.dma_start(WFX[:], wfx_d[:])
        WH, WFH = [], []
        for c in range(2):
            t = wpool.tile([P, 768], bf16, name=f"whs{c}")
            nc.sync.dma_start(t[:], wh_d[c])
            WH.append(t)
            t = wpool.tile([P, 256], bf16, name=f"wfhs{c}")
            nc.sync.dma_start(t[:], wfh_d[c])
            WFH.append(t)
        BIO = wpool.tile([P, 6], f32, name="bios")
        nc.sync.dma_start(BIO[:], bio_d[:])
        BF = wpool.tile([P, 2], f32, name="bfs")
        nc.sync.dma_start(BF[:], bf_d[:])

        # ---- persistent per-level H/C buffers [P, 2(feat), n] ----
        H = {l: hcpool.tile([P, 2, CNT[l]], bf16, name=f"H{l}") for l in LVLS}
        C = {l: hcpool.tile([P, 2, CNT[l]], bf16, name=f"C{l}") for l in LVLS}

        loop_cm = tc.For_i(0, repeat, 1) if repeat > 1 else nullcontext()

        def macro(lvl, j0):
            """process MACRO nodes at stored offset j0 of level lvl"""
            leaf = (lvl == 16)
            blk = (SEG[lvl] + j0) // MACRO
            HO, CO = H[lvl], C[lvl]
            xt = xpool.tile([P, 2, MACRO], fp8, name="xt", tag="xt")
            nc.sync.dma_start(xt[:], x_d[:, blk])

            if not leaf:
                HC, CC = H[lvl + 1], C[lvl + 1]
                jL, jR = j0, CNT[lvl] + j0
                hs = gpool.tile([P, 2, MACRO], bf16, tag="hs", bufs=2)
                nc.vector.tensor_add(hs[:], HC[:, :, jL:jL + MACRO],
                                     HC[:, :, jR:jR + MACRO])

            # ---- iou gates: 6 fo chunks of 128 feats x 1024 nodes ----
            gates = []
            for fo in range(6):
                pt = ppool.tile([P, 2, SUB], f32, name="pt", tag="iou")
                for s in range(2):
                    xs = xt[:, :, s * SUB:(s + 1) * SUB]
                    nc.tensor.matmul(pt[:, s, :],
                                     WX[:, :, fo * P:(fo + 1) * P], xs,
                                     start=True, stop=leaf, perf_mode=DR)
                    if not leaf:
                        sl = slice(s * SUB, (s + 1) * SUB)
                        nc.tensor.matmul(pt[:, s, :],
                                         WH[0][:, fo * P:(fo + 1) * P],
                                         hs[:, 0, sl], start=False,
                                         stop=False)
                        nc.tensor.matmul(pt[:, s, :],
                                         WH[1][:, fo * P:(fo + 1) * P],
                                         hs[:, 1, sl], start=False, stop=True)
                g = gpool.tile([P, 2, SUB], bf16, name="g", tag=f"g{fo}",
                               bufs=2)
                func = AF.Tanh if fo >= 4 else AF.Sigmoid
                nc.scalar.activation(g[:], pt[:], func, bias=BIO[:, fo:fo + 1])
                gates.append(g)

            # ---- forget gates + fc sum (internal only) ----
            if not leaf:
                fg = []
                for fo in range(2):
                    pf = fpool.tile([P, 2, 2, SUB], f32, name="pf", tag="fg")
                    for s in range(2):
                        xs = xt[:, :, s * SUB:(s + 1) * SUB]
                        for half, jc in ((0, jL), (1, jR)):
                            dst = pf[:, s, half, :]
                            nc.tensor.matmul(dst,
                                             WFX[:, :, fo * P:(fo + 1) * P],
                                             xs, start=True, stop=False,
                                             perf_mode=DR)
                            hsl = slice(jc + s * SUB, jc + (s + 1) * SUB)
                            nc.tensor.matmul(dst,
                                             WFH[0][:, fo * P:(fo + 1) * P],
                                             HC[:, 0, hsl], start=False,
                                             stop=False)
                            nc.tensor.matmul(dst,
                                             WFH[1][:, fo * P:(fo + 1) * P],
                                             HC[:, 1, hsl], start=False,
                                             stop=True)
                    g = gpool.tile([P, 2, 2, SUB], bf16, name="fg",
                                   tag=f"fg{fo}", bufs=2)
                    nc.scalar.activation(g[:], pf[:], AF.Sigmoid,
                                         bias=BF[:, fo:fo + 1])
                    fg.append(g)
                t1 = gpool.tile([P, 2, MACRO], bf16, tag="t1")
                t2 = gpool.tile([P, 2, MACRO], bf16, tag="t2")
                for f in range(2):
                    nc.vector.tensor_mul(t1[:, f, :], fg[f][:, :, 0, :],
                                         CC[:, f, jL:jL + MACRO])
                    nc.vector.tensor_mul(t2[:, f, :], fg[f][:, :, 1, :],
                                         CC[:, f, jR:jR + MACRO])
                t12 = gpool.tile([P, 2, MACRO], bf16, tag="t12")
                nc.vector.tensor_add(t12[:], t1[:], t2[:])

            # ---- c_new, tanh, h ----
            cs = CO[:, :, j0:j0 + MACRO]
            if leaf:
                for f in range(2):
                    nc.vector.tensor_mul(CO[:, f, j0:j0 + MACRO],
                                         gates[f][:], gates[4 + f][:])
            else:
                t3 = gpool.tile([P, 2, MACRO], bf16, tag="t3")
                for f in range(2):
                    nc.vector.tensor_mul(t3[:, f, :], gates[f][:],
                                         gates[4 + f][:])
                nc.vector.tensor_add(cs, t12[:], t3[:])
            th = gpool.tile([P, 2, MACRO], bf16, tag="th", bufs=2)
            nc.scalar.activation(th[:], cs, AF.Tanh)
            for f in range(2):
                nc.vector.tensor_mul(HO[:, f, j0:j0 + MACRO],
                                     gates[2 + f][:], th[:, f, :])

        with loop_cm:
            for lvl in LVLS:
                for j0 in range(0, CNT[lvl], MACRO):
                    macro(lvl, j0)
            nc.sync.dma_start(outh_d[:], H[CUT][:])
            nc.sync.dma_start(outc_d[:], C[CUT][:])

    nc.compile()
    _PROGRAM_CACHE[key] = nc
    return nc


def shard_inputs(inputs, W_ioux, b_ioux, W_iouh, b_iouh, W_fx, b_fx, W_fh,
                 b_fh):
    """Build per-core input maps."""
    from ml_dtypes import bfloat16, float8_e4m3
    so = stored_orders()
    f32 = np.float32

    def xside(w, rows):
        # [P, 2, rows]: w[p, h, m] = W[m, h*128+p], fp8
        a = np.asarray(w, f32).T.reshape(2, P, rows).transpose(1, 0, 2)
        return np.ascontiguousarray(np.clip(a, -240.0, 240.0)).astype(
            float8_e4m3)

    wioux = xside(W_ioux, 768)
    wfx = xside(W_fx, 256)
    wiouh = np.ascontiguousarray(
        np.asarray(W_iouh, f32).T.reshape(2, P, 768)).astype(bfloat16)
    wfh = np.ascontiguousarray(
        np.asarray(W_fh, f32).T.reshape(2, P, 256)).astype(bfloat16)
    bio = np.ascontiguousarray((np.asarray(b_ioux, f32)
                                + np.asarray(b_iouh, f32)).reshape(6, P).T)
    bf = np.ascontiguousarray((np.asarray(b_fx, f32)
                               + np.asarray(b_fh, f32)).reshape(2, P).T)
    inputs = np.asarray(inputs, f32)

    in_maps = []
    for k in range(NCORES):
        xk = np.empty((NLOC, D), dtype=f32)
        for l in LVLS:
            n = CNT[l]
            gs = 2 ** l - 1 + k * n
            xk[SEG[l]:SEG[l] + n] = inputs[gs:gs + n][so[l]]
        # [P, NBLK, 2, MACRO]: x8[p, b, h, j] = xk[b*MACRO+j, h*128+p]
        x8 = xk.T.reshape(2, P, NBLK, MACRO).transpose(1, 2, 0, 3)
        x8 = np.ascontiguousarray(np.clip(x8, -240.0, 240.0)).astype(
            float8_e4m3)
        in_maps.append({
            "x": x8, "wioux": wioux, "wiouh": wiouh, "wfx": wfx, "wfh": wfh,
            "bio": bio, "bf": bf,
        })
    return in_maps


def _sig(v):
    return 1.0 / (1.0 + np.exp(-v))


def top_of_tree(h_cut, c_cut, inputs, W_ioux, b_ioux, W_iouh, b_iouh,
                W_fx, b_fx, W_fh, b_fh):
    """numpy levels CUT-1..0. h_cut/c_cut: [2^CUT, 256] level-CUT states."""
    f32 = np.float32
    n_top = 2 ** (CUT + 1) - 1
    ncut = 2 ** CUT
    h = np.zeros((n_top, D), dtype=f32)
    c = np.zeros((n_top, D), dtype=f32)
    h[ncut - 1:] = h_cut
    c[ncut - 1:] = c_cut
    x = np.asarray(inputs[:ncut - 1], f32)
    iou_x = x @ np.asarray(W_ioux, f32).T + b_ioux
    fx = x @ np.asarray(W_fx, f32).T + b_fx
    W_iouh = np.asarray(W_iouh, f32)
    W_fh = np.asarray(W_fh, f32)

    for lvl in range(CUT - 1, -1, -1):
        start, count = 2 ** lvl - 1, 2 ** lvl
        cs = 2 * start + 1
        ch = h[cs:cs + 2 * count].reshape(count, 2, D)
        cc = c[cs:cs + 2 * count].reshape(count, 2, D)
        iou = iou_x[start:start + count] + ch.sum(axis=1) @ W_iouh.T + b_iouh
        f = _sig(np.einsum("nkm,pm->nkp", ch, W_fh, optimize=True) + b_fh
                 + fx[start:start + count][:, None, :])
        fc_sum = (f * cc).sum(axis=1)
        i, o, u = np.split(iou, 3, axis=1)
        c_new = _sig(i) * np.tanh(u) + fc_sum
        h_new = _sig(o) * np.tanh(c_new)
        c[start:start + count] = c_new
        h[start:start + count] = h_new
    return c[0:1].astype(f32), h[0:1].astype(f32)


def run_device(in_maps, trace=False, repeat=1):
    from concourse.bass_utils import run_bass_kernel_spmd
    nc = build_program(repeat)
    return run_bass_kernel_spmd(nc, in_maps, core_ids=list(range(NCORES)),
                                trace=trace)


def kernel(inputs, W_ioux, b_ioux, W_iouh, b_iouh, W_fx, b_fx, W_fh, b_fh):
    args = (inputs, W_ioux, b_ioux, W_iouh, b_iouh, W_fx, b_fx, W_fh, b_fh)
    in_maps = shard_inputs(*args)
    res = run_device(in_maps)
    f32 = np.float32
    so = stored_orders()[CUT]
    nt = CNT[CUT]
    ncut = 2 ** CUT
    h_cut = np.zeros((ncut, D), dtype=f32)
    c_cut = np.zeros((ncut, D), dtype=f32)
    for k in range(NCORES):
        oh = np.asarray(res.results[k]["outh"], f32)   # [P, 2, nt]
        oc = np.asarray(res.results[k]["outc"], f32)   # [P, 2, nt]
        idx = k * nt + so
        h_cut[idx] = oh.transpose(1, 0, 2).reshape(D, nt).T
        c_cut[idx] = oc.transpose(1, 0, 2).reshape(D, nt).T
    return top_of_tree(h_cut, c_cut, *args)
